# revision 27
# baseline (speedup 1.0000x reference)
"""Trainium2 Bass kernel: ExponentialConcordanceLoss over all pairs.

loss = sum_{i,j: d_i < d_j, e_i = 1} exp(p_j - p_i)  /  #{such pairs}

Strategy (8 NeuronCores, SPMD): shard the pairwise matrix by j — each core
owns 1024 j values and the full 8192 i range.  Using separability
exp(p_j - p_i) = exp(p_j) * exp(-p_i):

  per core:  s_j = sum_i [d_i < d_j] * (e_i * exp(-p_i))
             t_j = sum_i [d_i < d_j] * e_i
             partials = (sum_j exp(p_j) * s_j,  sum_j t_j)

The [d_i < d_j] comparison masks ([128 i x 1024 j] tiles, one per i-tile)
are generated on THREE engines concurrently:
  - Vector:  tensor_scalar is_gt (bf16 4x mode)      -> {0, 1}
  - GpSimd:  tensor_scalar is_gt                     -> {0, 1}
  - Scalar:  tanh(BIG*(d_j - d_i)), saturated        -> {-1, +1}
The masked sums run on the Tensor engine as matmuls with [c_i, e_i]
stationaries (M=2), packed 4-wide across PE column groups (tile_position).
Scalar-engine tiles use a 0.5x stationary so their contribution is
(target - 0.5*sum_tile(ce)); the constant deficit is added back in the
epilogue:  L += 0.5*C_act*G,  T += 0.5*E_act*J  where C_act/E_act are the
ce-sums over Scalar-assigned i-tiles and G = sum_j exp(p_j).

Per-core partials are summed on the host (a device AllReduce measures
~45us for 8 bytes on this fabric — pathological for an 8-byte reduce).

Implementation notes:
 - Every compute instruction may carry at most ONE new-semaphore sync wait;
   tiny "touch" ops absorb DMA/engine-crossing waits ahead of the hot ops.
 - tensor_tensor_reduce mis-executes on this runtime; epilogue uses
   copy + mul + reduce.
 - PSUM partitions outside the 4 column-group windows are zero-filled by an
   M=128 zero matmul; a min/max clamp additionally launders any residue
   before the fold matmul multiplies it by 0.
"""

import numpy as np
import ml_dtypes

N = 8192
NCORES = 8
P = 128
NT = N // P          # 64 i-tiles of 128
J = N // NCORES      # 1024 j per core
JC = 512             # matmul free-dim chunk
NG = 4               # PE column groups (tile_position packing)
BIG = float(2 ** 30)

# i-tile -> mask engine: Scalar(ACT, +-1 convention) for t%4==3 and t%16==2,
# Vector otherwise.  ACT-tile set must stay expressible as strided slices
# (the deficit correction reduces over those columns).
def _is_act_tile(t):
    return t % 4 == 3 or t % 16 == 2

_BF16 = ml_dtypes.bfloat16

_cached = None


def _build():
    from concourse import bacc, tile, mybir

    dt = mybir.dt
    Alu = mybir.AluOpType
    Act = mybir.ActivationFunctionType

    nc = bacc.Bacc("TRN2", target_bir_lowering=False, debug=False,
                   num_devices=NCORES)

    d_col = nc.dram_tensor("d_col", [P, NT], dt.float32, kind="ExternalInput").ap()
    p_col = nc.dram_tensor("p_col", [P, NT], dt.float32, kind="ExternalInput").ap()
    e_col = nc.dram_tensor("e_col", [P, NT], dt.float32, kind="ExternalInput").ap()
    dj_bc = nc.dram_tensor("dj_bcast", [P, J], dt.bfloat16, kind="ExternalInput").ap()
    pj_row = nc.dram_tensor("pj_row", [1, J], dt.float32, kind="ExternalInput").ap()
    fold_i = nc.dram_tensor("fold", [P, 2], dt.float32, kind="ExternalInput").ap()
    out_d = nc.dram_tensor("out", [1, 2], dt.float32, kind="ExternalOutput").ap()

    with tile.TileContext(nc) as tc:
        with (
            tc.tile_pool(name="cpool", bufs=1) as cpool,
            # One slot per mask tile: slot reuse would add a second
            # sync-wait to the generating op (only one allowed).
            tc.tile_pool(name="mpool", bufs=NT) as mpool,
            tc.tile_pool(name="pspool", bufs=1, space="PSUM") as pspool,
        ):
            # ---- input loads, spread across engine DMA queues
            dj_sb = cpool.tile([P, J], dt.bfloat16)
            dma_engines = [nc.sync, nc.gpsimd, nc.scalar, nc.sync]
            for q in range(4):
                pr = slice(32 * q, 32 * (q + 1))
                dma_engines[q].dma_start(dj_sb[pr, :], dj_bc[pr, :])
            dcol_sb = cpool.tile([P, NT], dt.float32)
            nc.gpsimd.dma_start(dcol_sb[:], d_col[:])
            pcol_sb = cpool.tile([P, NT], dt.float32)
            nc.gpsimd.dma_start(pcol_sb[:], p_col[:])
            ecol_sb = cpool.tile([P, NT], dt.float32)
            nc.sync.dma_start(ecol_sb[:], e_col[:])
            pj_sb = cpool.tile([1, J], dt.float32)
            nc.sync.dma_start(pj_sb[:], pj_row[:])
            fold_sb = cpool.tile([P, 2], dt.float32)
            nc.sync.dma_start(fold_sb[:], fold_i[:])

            # DVE-owned copies: absorb the DMA waits AND give downstream
            # PE/ACT consumers a DVE-only dependency.
            fold_cp = cpool.tile([P, 2], dt.float32)
            nc.vector.tensor_copy(fold_cp[:], fold_sb[:])
            dj_act = cpool.tile([P, J], dt.bfloat16)   # for ACT mask reads

            # ---- DVE touches: absorb one DMA-queue wait each
            scratch = cpool.tile([1, 12], dt.float32)
            for q in range(4):
                nc.vector.tensor_copy(scratch[0:1, q:q + 1],
                                      dj_sb[32 * q:32 * q + 1, 0:1])
            nc.vector.tensor_copy(scratch[0:1, 4:5], dcol_sb[0:1, 0:1])
            nc.vector.tensor_copy(scratch[0:1, 5:6], ecol_sb[0:1, 0:1])
            nc.vector.tensor_copy(dj_act[:], dj_sb[:])
            # ACT touches (Copy keeps bias immediate -> no const-AP dep)
            scratch_a = cpool.tile([1, 2], dt.float32)
            nc.scalar.activation(scratch_a[0:1, 0:1], pj_sb[0:1, 0:1], Act.Copy)
            nc.scalar.activation(scratch_a[0:1, 1:2], pcol_sb[0:1, 0:1], Act.Copy)

            # ---- c_i = e_i * exp(-p_i); per-i-tile stationary [c | e] bf16
            expnp = cpool.tile([P, NT], dt.float32)
            nc.scalar.activation(expnp[:], pcol_sb[:], Act.Exp, scale=-1.0)
            ccol = cpool.tile([P, NT], dt.float32)
            nc.vector.tensor_mul(ccol[:], expnp[:], ecol_sb[:])
            ce = cpool.tile([P, NT, 2], dt.bfloat16)
            nc.vector.tensor_copy(ce[:, :, 0], ccol[:])
            nc.vector.tensor_copy(ce[:, :, 1], ecol_sb[:])
            # 0.5x stationary for the +-1 (ACT) tiles: exact in bf16
            ceh = cpool.tile([P, NT, 2], dt.bfloat16)
            nc.vector.tensor_scalar(ceh[:, :, :], ce[:, :, :], 0.5, None,
                                    Alu.mult)
            # ACT mask bias: -BIG * d_i
            dbig = cpool.tile([P, NT], dt.float32)
            nc.vector.tensor_scalar(dbig[:], dcol_sb[:], -BIG, None, Alu.mult)

            # ---- j-side weights replicated per column group:
            # w4[32g+0, :] = exp(p_j), w4[32g+1, :] = 1, 0 elsewhere
            # G = sum_j exp(p_j) falls out of the exp's accumulator.
            w4 = cpool.tile([P, J], dt.float32)
            gsum = cpool.tile([1, 1], dt.float32)
            nc.vector.memset(w4[:], 0.0)
            nc.vector.memset(w4[0:2, :], 1.0)
            nc.scalar.activation(w4[0:1, :], pj_sb[:], Act.Exp,
                                 accum_out=gsum[:])
            nc.vector.tensor_copy(scratch[0:1, 6:7], w4[0:1, 0:1])
            for g in range(1, NG):
                nc.sync.dma_start(w4[32 * g:32 * g + 2, :], w4[0:2, :])
            for g in range(1, NG):
                nc.vector.tensor_copy(scratch[0:1, 6 + g:7 + g],
                                      w4[32 * g:32 * g + 1, 0:1])

            # ---- ACT-tile ce sums (for the +-1 deficit correction)
            cae = cpool.tile([P, 2], dt.float32)
            cae_b = cpool.tile([P, 2], dt.float32)
            for k, cols in enumerate((slice(3, NT, 4), slice(2, NT, 16))):
                dst = cae if k == 0 else cae_b
                nc.vector.tensor_reduce(dst[:, 0:1], ccol[:, cols],
                                        mybir.AxisListType.X, Alu.add)
                nc.vector.tensor_reduce(dst[:, 1:2], ecol_sb[:, cols],
                                        mybir.AxisListType.X, Alu.add)
            nc.vector.tensor_add(cae[:], cae[:], cae_b[:])
            ones128 = cpool.tile([P, 1], dt.float32)
            nc.vector.memset(ones128[:], 1.0)

            # ---- pairwise masks + col-tiled matmul accumulation
            nchunk = J // JC
            ps = [pspool.tile([P, JC], dt.float32, name=f"ps{c}")
                  for c in range(nchunk)]
            # zero-fill the full PSUM tiles (M=128 zero matmul) so the
            # never-matmul'd partitions read back as 0.0
            zt = cpool.tile([P, JC], dt.bfloat16)
            nc.vector.memset(zt[:], 0.0)
            for c in range(nchunk):
                nc.tensor.matmul(ps[c][:], zt[:, 0:P], zt[:],
                                 start=True, stop=False, skip_group_check=True)
            for t in range(NT):
                g = t % NG
                pr = slice(32 * g, 32 * g + 2)
                mask = mpool.tile([P, J], dt.bfloat16, tag="mask", name="mask")
                if not _is_act_tile(t):
                    nc.vector.tensor_scalar(
                        mask[:], dj_sb[:], dcol_sb[:, t:t + 1], None, Alu.is_gt)
                    stat = ce
                else:
                    nc.scalar.activation(
                        mask[:], dj_act[:], Act.Tanh,
                        bias=dbig[:, t:t + 1], scale=BIG)
                    stat = ceh
                for c in range(nchunk):
                    nc.tensor.matmul(
                        ps[c][pr, :], stat[:, t, :],
                        mask[:, c * JC:(c + 1) * JC],
                        start=False, stop=(t >= NT - NG),
                        skip_group_check=True,
                        tile_position=(0, 32 * g))

            # ---- epilogue: fold the 4 groups, reduce over j, correct
            st4 = cpool.tile([P, J], dt.float32)
            nc.vector.tensor_copy(st4[:, 0:JC], ps[0][:])
            nc.scalar.activation(st4[:, JC:J], ps[1][:], Act.Copy)
            prod4 = cpool.tile([P, J], dt.float32)
            nc.vector.tensor_mul(prod4[:], st4[:], w4[:])
            red4 = cpool.tile([P, 1], dt.float32)
            nc.scalar.activation(st4[:, 0:J], prod4[:], Act.Copy,
                                 accum_out=red4[:])
            red4c = cpool.tile([P, 1], dt.float32)
            nc.vector.tensor_scalar(red4c[:], red4[:], -1e30, 1e30,
                                    Alu.max, Alu.min)
            ps_f = pspool.tile([2, 1], dt.float32)
            nc.tensor.matmul(ps_f[:], fold_cp[:], red4c[:],
                             start=True, stop=True)
            ps_ce = pspool.tile([2, 1], dt.float32)
            nc.tensor.matmul(ps_ce[:], cae[:], ones128[:],
                             start=True, stop=True)
            red = cpool.tile([2, 1], dt.float32)
            nc.vector.tensor_copy(red[:], ps_f[:])
            ce2 = cpool.tile([2, 1], dt.float32)
            nc.vector.tensor_copy(ce2[:], ps_ce[:])
            # corr = 0.5 * [C_act * G ; E_act * J]
            gj2 = cpool.tile([2, 1], dt.float32)
            nc.vector.memset(gj2[:], float(J))
            nc.vector.tensor_copy(gj2[0:1, 0:1], gsum[:])
            corr = cpool.tile([2, 1], dt.float32)
            nc.vector.tensor_mul(corr[:], ce2[:], gj2[:])
            corrh = cpool.tile([2, 1], dt.float32)
            nc.vector.tensor_scalar(corrh[:], corr[:], 0.5, None, Alu.mult)
            redf = cpool.tile([2, 1], dt.float32)
            nc.vector.tensor_add(redf[:], red[:], corrh[:])
            # emit the per-core partials; host reduces across cores
            nc.sync.dma_start(out_d[0:1, 0:2], redf[0:2, 0:1])

    nc.finalize()
    return nc


def _get_program():
    global _cached
    if _cached is None:
        _cached = _build()
    return _cached


def _reduce_output(results):
    parts = np.stack([np.asarray(r["out"], dtype=np.float64).reshape(2)
                      for r in results])
    tot = parts.sum(axis=0)
    return np.float32(tot[0] / tot[1]).reshape(())


def _shard_inputs(preds, targets):
    p = np.ascontiguousarray(np.asarray(preds, dtype=np.float32).reshape(-1))
    d = np.ascontiguousarray(np.asarray(targets[:, 0], dtype=np.float32))
    e = np.ascontiguousarray(np.asarray(targets[:, 1], dtype=np.float32))

    d_col = np.ascontiguousarray(d.reshape(NT, P).T)
    p_col = np.ascontiguousarray(p.reshape(NT, P).T)
    e_col = np.ascontiguousarray(e.reshape(NT, P).T)
    fold = np.zeros((P, 2), dtype=np.float32)
    for g in range(NG):
        fold[32 * g + 0, 0] = 1.0
        fold[32 * g + 1, 1] = 1.0

    in_maps = []
    for k in range(NCORES):
        sl = slice(J * k, J * (k + 1))
        dj = d[sl].astype(_BF16)
        in_maps.append({
            "d_col": d_col,
            "p_col": p_col,
            "e_col": e_col,
            "dj_bcast": np.ascontiguousarray(
                np.broadcast_to(dj[None, :], (P, J))),
            "pj_row": np.ascontiguousarray(p[sl].reshape(1, J)),
            "fold": fold,
        })
    return in_maps


def _run(preds, targets, trace=False):
    from concourse import bass_utils

    nc = _get_program()
    in_maps = _shard_inputs(preds, targets)
    res = bass_utils.run_bass_kernel_spmd(
        nc, in_maps, list(range(NCORES)), trace=trace)
    out = _reduce_output(res.results)
    return out, res


def kernel(preds, targets):
    out, _ = _run(preds, targets, trace=False)
    return out


def kernel_traced(preds, targets):
    """Returns (loss, BassKernelResults) with NTFF profiling enabled."""
    return _run(preds, targets, trace=True)
